# revision 16
# baseline (speedup 1.0000x reference)
"""AttentionReadout (segment softmax attention pooling) on 8 Trainium2 cores.

Math (reference):
    logits = tanh(x @ W1 + b1) @ W2 + b2          # [N, 4]
    attn   = segment_softmax(logits, batch)       # [N, 4]
    xt     = x @ Wt + bt                          # [N, 256] -> [N, 4, 64]
    graph_emb = segment_sum(attn[:, :, None] * xt)  # [1024, 256]
    returns (graph_emb, attn)

Strategy:
  * Host packs nodes into a segment-padded layout: every segment gets L
    (>=512, multiple of 128) node slots, padding rows are zero and carry a
    -30000 logit mask.  1024 segments / 8 cores = 128 whole segments per
    core -> segment reductions are device-local, fixed-shape free-dim
    reductions.
  * Host pre-transposes x per segment to [hidden, L] so every matmul uses
    the weights as natural lhsT ([K, M]) and the activations as rhs
    ([K, N=L]) -- no on-device transposes.
  * All matmuls run as float32r (full fp32 data, 1 cycle/row at N>=256).
  * exp+segment-sum fuse into one scalar-engine activation (accum_out);
    (xt + bt) * attn and the graph sum fuse into one DVE
    scalar_tensor_tensor with accum_out.
  * Outputs: attn in transposed padded layout [nseg, 4, L]; graph
    embeddings as [2, 128, nseg] columns.  Host scatters back.
"""

import numpy as np
from contextlib import ExitStack

import concourse.bass as bass
import concourse.bacc as bacc
import concourse.tile as tile
from concourse import mybir
from concourse.bass_utils import run_bass_kernel_spmd

N_CORES = 8
H = 256
NUM_HEADS = 4
HEAD_DIM = H // NUM_HEADS
B = 1024
NSEG = B // N_CORES  # segments per core
NEG_MASK = -30000.0
GS = 16  # segments per softmax batch group

F32 = mybir.dt.float32
F32R = mybir.dt.float32r
AX = mybir.AxisListType.X
ALU = mybir.AluOpType
ACT = mybir.ActivationFunctionType

# compiled program cache, keyed by L (pad length per segment)
_programs: dict[int, bass.Bass] = {}

LAST_EXEC_NS = None
LAST_RESULT = None
TRACE = False


def _build_program(L: int, nseg: int = NSEG, gs: int = GS) -> bass.Bass:
    ngroups = nseg // gs
    R = NUM_HEADS * gs  # stacked softmax rows per group
    nc = bacc.Bacc()

    xpt = nc.dram_tensor("xpt", [nseg, 2, 128, L], F32R, kind="ExternalInput")
    maskt = nc.dram_tensor("maskt", [nseg, NUM_HEADS, L], F32, kind="ExternalInput")
    w1 = nc.dram_tensor("w1", [H, H], F32R, kind="ExternalInput")
    w2 = nc.dram_tensor("w2", [H, NUM_HEADS], F32R, kind="ExternalInput")
    wt = nc.dram_tensor("wt", [H, H], F32R, kind="ExternalInput")
    b1d = nc.dram_tensor("b1d", [H, 1], F32, kind="ExternalInput")
    b2d = nc.dram_tensor("b2d", [R, 1], F32, kind="ExternalInput")
    btd = nc.dram_tensor("btd", [H, 1], F32, kind="ExternalInput")
    e4d = nc.dram_tensor("e4d", [gs, 2, R, 128], F32R, kind="ExternalInput")

    attn_out = nc.dram_tensor(
        "attn_out", [ngroups, R, L], F32R, kind="ExternalOutput"
    )
    g_out = nc.dram_tensor("g_out", [2, 128, nseg], F32, kind="ExternalOutput")

    with tile.TileContext(nc) as tc, ExitStack() as ctx:
        const = ctx.enter_context(tc.tile_pool(name="const", bufs=1))
        sb = ctx.enter_context(tc.tile_pool(name="sb", bufs=2))
        ps = ctx.enter_context(tc.tile_pool(name="ps", bufs=1, space="PSUM"))

        # --- persistent weights / constants ---
        w1s0 = const.tile([128, H], F32R, tag="w1s0", name="w1s0")
        w1s1 = const.tile([128, H], F32R, tag="w1s1", name="w1s1")
        nc.sync.dma_start(w1s0[:], w1[0:128, :])
        nc.sync.dma_start(w1s1[:], w1[128:256, :])
        wts0 = const.tile([128, H], F32R, tag="wts0", name="wts0")
        wts1 = const.tile([128, H], F32R, tag="wts1", name="wts1")
        nc.sync.dma_start(wts0[:], wt[0:128, :])
        nc.sync.dma_start(wts1[:], wt[128:256, :])
        w2s0 = const.tile([128, NUM_HEADS], F32R, tag="w2s0", name="w2s0")
        w2s1 = const.tile([128, NUM_HEADS], F32R, tag="w2s1", name="w2s1")
        nc.sync.dma_start(w2s0[:], w2[0:128, :])
        nc.sync.dma_start(w2s1[:], w2[128:256, :])
        b1s = const.tile([128, 2], F32, tag="b1s", name="b1s")
        nc.sync.dma_start(b1s[:, 0:1], b1d[0:128, :])
        nc.sync.dma_start(b1s[:, 1:2], b1d[128:256, :])
        bts = const.tile([128, 2], F32, tag="bts", name="bts")
        nc.sync.dma_start(bts[:, 0:1], btd[0:128, :])
        nc.sync.dma_start(bts[:, 1:2], btd[128:256, :])
        b2s = const.tile([R, 1], F32, tag="b2s", name="b2s")
        nc.sync.dma_start(b2s[:], b2d[:, :])
        e4s = const.tile([R, gs * 2 * 128], F32R, tag="e4s", name="e4s")
        for k in range(gs):
            for j in range(2):
                blk = (k * 2 + j) * 128
                nc.sync.dma_start(e4s[:, blk : blk + 128], e4d[k, j, :, :])

        # graph embedding accumulators: column s = segment s
        g0 = const.tile([128, nseg], F32, tag="g0", name="g0")
        g1 = const.tile([128, nseg], F32, tag="g1", name="g1")

        w1s = [w1s0, w1s1]
        wts = [wts0, wts1]
        w2s = [w2s0, w2s1]

        for g in range(ngroups):
            # masked logits of gs segments stacked on partitions:
            # row 4k+h = (seg k, head h)
            lms = sb.tile([R, L], F32, tag="lms", name="lms")

            xts_all = []
            for k in range(gs):
                s = g * gs + k
                xp0 = sb.tile([128, L], F32R, tag="xp0", bufs=3, name="xp0")
                xp1 = sb.tile([128, L], F32R, tag="xp1", bufs=3, name="xp1")
                nc.sync.dma_start(xp0[:], xpt[s, 0, :, :])
                nc.sync.dma_start(xp1[:], xpt[s, 1, :, :])
                xps = [xp0, xp1]

                # hT = tanh(W1^T x^T + b1)
                hs = []
                for j in range(2):
                    ph = ps.tile([128, L], F32, tag=f"ph{j}", name=f"ph{j}")
                    for i in range(2):
                        nc.tensor.matmul(
                            ph[:],
                            w1s[i][:, j * 128 : (j + 1) * 128],
                            xps[i][:],
                            start=(i == 0),
                            stop=(i == 1),
                        )
                    hj = sb.tile([128, L], F32R, tag=f"h{j}", name=f"h{j}")
                    nc.scalar.activation(
                        hj[:], ph[:], ACT.Tanh, bias=b1s[:, j : j + 1]
                    )
                    hs.append(hj)

                # logits -> psum [4, L]; drain+mask on DVE; DMA-stack rows
                pl = ps.tile([NUM_HEADS, L], F32, tag="pl", bufs=2, name="pl")
                for i in range(2):
                    nc.tensor.matmul(
                        pl[:], w2s[i][:], hs[i][:], start=(i == 0), stop=(i == 1)
                    )
                mk = sb.tile([NUM_HEADS, L], F32, tag="mk", bufs=3, name="mk")
                nc.sync.dma_start(mk[:], maskt[s, :, :])
                lml = sb.tile([NUM_HEADS, L], F32, tag="lml", bufs=3, name="lml")
                nc.vector.tensor_add(lml[:], pl[:], mk[:])
                nc.sync.dma_start(lms[4 * k : 4 * k + 4, :], lml[:])

                # xt^T = Wt^T x^T; drain to SBUF with bias fused (DVE)
                xts_pair = []
                for j in range(2):
                    px = ps.tile([128, L], F32, tag=f"px{j}", name=f"px{j}")
                    for i in range(2):
                        nc.tensor.matmul(
                            px[:],
                            wts[i][:, j * 128 : (j + 1) * 128],
                            xps[i][:],
                            start=(i == 0),
                            stop=(i == 1),
                        )
                    xts = sb.tile(
                        [128, L], F32, tag=f"xts{j}", bufs=gs + 2, name=f"xts{j}"
                    )
                    nc.vector.tensor_scalar_add(xts[:], px[:], bts[:, j : j + 1])
                    xts_pair.append(xts)
                xts_all.append(xts_pair)

            # ---- batched segment softmax over the whole group ----
            nmax = sb.tile([R, 1], F32, tag="nmax", name="nmax")
            nc.vector.reduce_max(nmax[:], lms[:], axis=AX, negate=True)
            ebias = sb.tile([R, 1], F32, tag="ebias", name="ebias")
            nc.vector.tensor_add(ebias[:], nmax[:], b2s[:])
            ev = sb.tile([R, L], F32, tag="ev", name="ev")
            ssum = sb.tile([R, 1], F32, tag="ssum", name="ssum")
            nc.scalar.activation(
                ev[:], lms[:], ACT.Exp, bias=ebias[:, 0:1], accum_out=ssum[:]
            )
            rinv = sb.tile([R, 1], F32, tag="rinv", name="rinv")
            nc.vector.reciprocal(rinv[:], ssum[:])
            at = sb.tile([R, L], F32R, tag="at", name="at")
            nc.vector.tensor_scalar_mul(at[:], ev[:], rinv[:, 0:1])
            nc.sync.dma_start(attn_out[g, :, :], at[:])

            # ---- per segment: replicate attn, weight, graph-sum ----
            for k in range(gs):
                s = g * gs + k
                for j in range(2):
                    blk = (k * 2 + j) * 128
                    rep = ps.tile([128, L], F32, tag=f"rep{j}", name=f"rep{j}")
                    nc.tensor.matmul(
                        rep[:],
                        e4s[:, blk : blk + 128],
                        at[:],
                        start=True,
                        stop=True,
                    )
                    wsc = sb.tile([128, L], F32, tag=f"wsc{j}", name=f"wsc{j}")
                    gcol = (g0 if j == 0 else g1)[:, s : s + 1]
                    nc.vector.scalar_tensor_tensor(
                        wsc[:],
                        xts_all[k][j][:],
                        1.0,
                        rep[:],
                        op0=ALU.mult,
                        op1=ALU.mult,
                        accum_out=gcol,
                    )

        nc.sync.dma_start(g_out[0, :, :], g0[:])
        nc.sync.dma_start(g_out[1, :, :], g1[:])

    nc.finalize()
    return nc


def kernel(x, batch, W1, b1, W2, b2, Wt, bt):
    global LAST_EXEC_NS, LAST_RESULT

    x = np.ascontiguousarray(np.asarray(x, dtype=np.float32))
    batch = np.asarray(batch)
    N = x.shape[0]

    starts = np.searchsorted(batch, np.arange(B + 1)).astype(np.int64)
    lens = np.diff(starts)
    maxlen = int(lens.max())
    L = max(512, int(np.ceil(maxlen / 128.0)) * 128)

    if L not in _programs:
        _programs[L] = _build_program(L)
    nc = _programs[L]
    ngroups = NSEG // GS

    # node -> (segment, position-within-segment)
    pos = np.arange(N, dtype=np.int64) - starts[batch]

    # padded, per-segment-transposed x: [B, H, L]
    xp = np.zeros((B, L, H), dtype=np.float32)
    xp[batch, pos] = x
    xpt_all = np.ascontiguousarray(xp.transpose(0, 2, 1))  # [B, H, L]
    del xp

    mask1 = np.full((B, L), NEG_MASK, dtype=np.float32)
    mask1[batch, pos] = 0.0
    mask_all = np.ascontiguousarray(
        np.broadcast_to(mask1[:, None, :], (B, NUM_HEADS, L))
    )

    W1 = np.ascontiguousarray(np.asarray(W1, dtype=np.float32))
    W2 = np.ascontiguousarray(np.asarray(W2, dtype=np.float32))
    Wt = np.ascontiguousarray(np.asarray(Wt, dtype=np.float32))
    b1c = np.ascontiguousarray(np.asarray(b1, dtype=np.float32).reshape(H, 1))
    b2c = np.ascontiguousarray(
        np.tile(np.asarray(b2, dtype=np.float32).reshape(1, NUM_HEADS), (GS, 1)).reshape(
            GS * NUM_HEADS, 1
        )
    )
    e4 = np.zeros((GS, 2, GS * NUM_HEADS, 128), dtype=np.float32)
    for k in range(GS):
        for j in range(2):
            for hh in range(2):
                e4[k, j, 4 * k + 2 * j + hh, hh * HEAD_DIM : (hh + 1) * HEAD_DIM] = 1.0
    btc = np.ascontiguousarray(np.asarray(bt, dtype=np.float32).reshape(H, 1))

    in_maps = []
    for c in range(N_CORES):
        seg_lo = c * NSEG
        xpt_c = np.ascontiguousarray(
            xpt_all[seg_lo : seg_lo + NSEG].reshape(NSEG, 2, 128, L)
        )
        in_maps.append(
            dict(
                xpt=xpt_c,
                maskt=np.ascontiguousarray(mask_all[seg_lo : seg_lo + NSEG]),
                w1=W1,
                w2=W2,
                wt=Wt,
                b1d=b1c,
                b2d=b2c,
                e4d=e4,
                btd=btc,
            )
        )

    res = run_bass_kernel_spmd(nc, in_maps, list(range(N_CORES)), trace=TRACE)
    LAST_RESULT = res
    LAST_EXEC_NS = res.exec_time_ns

    # ---- unpack ----
    graph_emb = np.empty((B, H), dtype=np.float32)
    attn_all = np.empty((B, NUM_HEADS, L), dtype=np.float32)
    for c in range(N_CORES):
        seg_lo = c * NSEG
        g = res.results[c]["g_out"]  # [2, 128, NSEG]
        graph_emb[seg_lo : seg_lo + NSEG] = g.transpose(2, 0, 1).reshape(NSEG, H)
        attn_all[seg_lo : seg_lo + NSEG] = res.results[c]["attn_out"].reshape(
            NSEG, NUM_HEADS, L
        )

    graph_emb[lens == 0] = 0.0
    attn = attn_all[batch, :, pos]  # [N, 4]
    return graph_emb, attn


# revision 18
# speedup vs baseline: 1.1993x; 1.1993x over previous
"""AttentionReadout (segment softmax attention pooling) on 8 Trainium2 cores.

Math (reference):
    logits = tanh(x @ W1 + b1) @ W2 + b2          # [N, 4]
    attn   = segment_softmax(logits, batch)       # [N, 4]
    xt     = x @ Wt + bt                          # [N, 256] -> [N, 4, 64]
    graph_emb = segment_sum(attn[:, :, None] * xt)  # [1024, 256]
    returns (graph_emb, attn)

Strategy:
  * Host packs nodes into a segment-padded layout: every segment gets L
    (>=512, multiple of 128) node slots, padding rows are zero and carry a
    -30000 logit mask.  1024 segments / 8 cores = 128 whole segments per
    core -> segment reductions are device-local, fixed-shape free-dim
    reductions.
  * Host pre-transposes x per segment to [hidden, L] so every matmul uses
    the weights as natural lhsT ([K, M]) and the activations as rhs
    ([K, N=L]) -- no on-device transposes.
  * All matmuls run as float32r (full fp32 data, 1 cycle/row at N>=256).
  * exp+segment-sum fuse into one scalar-engine activation (accum_out);
    (xt + bt) * attn and the graph sum fuse into one DVE
    scalar_tensor_tensor with accum_out.
  * Outputs: attn in transposed padded layout [nseg, 4, L]; graph
    embeddings as [2, 128, nseg] columns.  Host scatters back.
"""

import numpy as np
from contextlib import ExitStack

import concourse.bass as bass
import concourse.bacc as bacc
import concourse.tile as tile
from concourse import mybir
from concourse.bass_utils import run_bass_kernel_spmd

N_CORES = 8
H = 256
NUM_HEADS = 4
HEAD_DIM = H // NUM_HEADS
B = 1024
NSEG = B // N_CORES  # segments per core
NEG_MASK = -30000.0
GS = 8  # segments per softmax batch group

F32 = mybir.dt.float32
F32R = mybir.dt.float32r
AX = mybir.AxisListType.X
ALU = mybir.AluOpType
ACT = mybir.ActivationFunctionType

# compiled program cache, keyed by L (pad length per segment)
_programs: dict[int, bass.Bass] = {}

LAST_EXEC_NS = None
LAST_RESULT = None
TRACE = False


def _build_program(L: int, nseg: int = NSEG, gs: int = GS) -> bass.Bass:
    ngroups = nseg // gs
    R = NUM_HEADS * gs  # stacked softmax rows per group
    nc = bacc.Bacc()

    xpt = nc.dram_tensor("xpt", [nseg, 2, 128, L], F32R, kind="ExternalInput")
    maskt = nc.dram_tensor("maskt", [nseg, NUM_HEADS, L], F32, kind="ExternalInput")
    w1 = nc.dram_tensor("w1", [H, H], F32R, kind="ExternalInput")
    w2 = nc.dram_tensor("w2", [H, NUM_HEADS], F32R, kind="ExternalInput")
    wt = nc.dram_tensor("wt", [H, H], F32R, kind="ExternalInput")
    b1d = nc.dram_tensor("b1d", [H, 1], F32, kind="ExternalInput")
    b2d = nc.dram_tensor("b2d", [R, 1], F32, kind="ExternalInput")
    btd = nc.dram_tensor("btd", [H, 1], F32, kind="ExternalInput")
    e4d = nc.dram_tensor("e4d", [gs, 2, R, 128], F32R, kind="ExternalInput")

    attn_out = nc.dram_tensor(
        "attn_out", [ngroups, R, L], F32R, kind="ExternalOutput"
    )
    g_out = nc.dram_tensor("g_out", [2, 128, nseg], F32, kind="ExternalOutput")

    with tile.TileContext(nc) as tc, ExitStack() as ctx:
        const = ctx.enter_context(tc.tile_pool(name="const", bufs=1))
        sb = ctx.enter_context(tc.tile_pool(name="sb", bufs=2))
        ps = ctx.enter_context(tc.tile_pool(name="ps", bufs=1, space="PSUM"))

        # --- persistent weights / constants ---
        w1s0 = const.tile([128, H], F32R, tag="w1s0", name="w1s0")
        w1s1 = const.tile([128, H], F32R, tag="w1s1", name="w1s1")
        nc.sync.dma_start(w1s0[:], w1[0:128, :])
        nc.sync.dma_start(w1s1[:], w1[128:256, :])
        wts0 = const.tile([128, H], F32R, tag="wts0", name="wts0")
        wts1 = const.tile([128, H], F32R, tag="wts1", name="wts1")
        nc.sync.dma_start(wts0[:], wt[0:128, :])
        nc.sync.dma_start(wts1[:], wt[128:256, :])
        w2s0 = const.tile([128, NUM_HEADS], F32R, tag="w2s0", name="w2s0")
        w2s1 = const.tile([128, NUM_HEADS], F32R, tag="w2s1", name="w2s1")
        nc.sync.dma_start(w2s0[:], w2[0:128, :])
        nc.sync.dma_start(w2s1[:], w2[128:256, :])
        b1s = const.tile([128, 2], F32, tag="b1s", name="b1s")
        nc.sync.dma_start(b1s[:, 0:1], b1d[0:128, :])
        nc.sync.dma_start(b1s[:, 1:2], b1d[128:256, :])
        bts = const.tile([128, 2], F32, tag="bts", name="bts")
        nc.sync.dma_start(bts[:, 0:1], btd[0:128, :])
        nc.sync.dma_start(bts[:, 1:2], btd[128:256, :])
        b2s = const.tile([R, 1], F32, tag="b2s", name="b2s")
        nc.sync.dma_start(b2s[:], b2d[:, :])
        e4s = const.tile([R, gs * 2 * 128], F32R, tag="e4s", name="e4s")
        for k in range(gs):
            for j in range(2):
                blk = (k * 2 + j) * 128
                nc.sync.dma_start(e4s[:, blk : blk + 128], e4d[k, j, :, :])

        # graph embedding accumulators: column s = segment s
        g0 = const.tile([128, nseg], F32, tag="g0", name="g0")
        g1 = const.tile([128, nseg], F32, tag="g1", name="g1")

        w1s = [w1s0, w1s1]
        wts = [wts0, wts1]
        w2s = [w2s0, w2s1]

        for g in range(ngroups):
            # masked logits of gs segments stacked on partitions:
            # row 4k+h = (seg k, head h)
            lms = sb.tile([R, L], F32, tag="lms", name="lms")

            xts_all = []
            for k in range(gs):
                s = g * gs + k
                xp0 = sb.tile([128, L], F32R, tag="xp0", bufs=3, name="xp0")
                xp1 = sb.tile([128, L], F32R, tag="xp1", bufs=3, name="xp1")
                nc.sync.dma_start(xp0[:], xpt[s, 0, :, :])
                nc.sync.dma_start(xp1[:], xpt[s, 1, :, :])
                xps = [xp0, xp1]

                # hT = tanh(W1^T x^T + b1)
                hs = []
                for j in range(2):
                    ph = ps.tile([128, L], F32, tag=f"ph{j}", name=f"ph{j}")
                    for i in range(2):
                        nc.tensor.matmul(
                            ph[:],
                            w1s[i][:, j * 128 : (j + 1) * 128],
                            xps[i][:],
                            start=(i == 0),
                            stop=(i == 1),
                        )
                    hj = sb.tile([128, L], F32R, tag=f"h{j}", name=f"h{j}")
                    nc.scalar.activation(
                        hj[:], ph[:], ACT.Tanh, bias=b1s[:, j : j + 1]
                    )
                    hs.append(hj)

                # logits -> psum [4, L]; DVE drain; gpsimd memset mask; stack
                pl = ps.tile([NUM_HEADS, L], F32, tag="pl", bufs=2, name="pl")
                for i in range(2):
                    nc.tensor.matmul(
                        pl[:], w2s[i][:], hs[i][:], start=(i == 0), stop=(i == 1)
                    )
                mk = sb.tile([NUM_HEADS, L], F32, tag="mk", bufs=3, name="mk")
                nc.sync.dma_start(mk[:], maskt[s, :, :])
                lml = sb.tile([NUM_HEADS, L], F32, tag="lml", bufs=3, name="lml")
                nc.vector.tensor_add(lml[:], pl[:], mk[:])
                nc.sync.dma_start(lms[4 * k : 4 * k + 4, :], lml[:])

                # xt^T = Wt^T x^T; drain psum->sbuf on ACT with bias fused
                xts_pair = []
                for j in range(2):
                    px = ps.tile([128, L], F32, tag=f"px{j}", name=f"px{j}")
                    for i in range(2):
                        nc.tensor.matmul(
                            px[:],
                            wts[i][:, j * 128 : (j + 1) * 128],
                            xps[i][:],
                            start=(i == 0),
                            stop=(i == 1),
                        )
                    xts = sb.tile(
                        [128, L], F32, tag=f"xts{j}", bufs=2 * gs + 2,
                        name=f"xts{j}",
                    )
                    nc.scalar.activation(
                        xts[:], px[:], ACT.Identity, bias=bts[:, j : j + 1]
                    )
                    xts_pair.append(xts)
                xts_all.append(xts_pair)

            # ---- batched segment softmax over the whole group ----
            nmax = sb.tile([R, 1], F32, tag="nmax", name="nmax")
            nc.vector.reduce_max(nmax[:], lms[:], axis=AX, negate=True)
            ebias = sb.tile([R, 1], F32, tag="ebias", name="ebias")
            nc.vector.tensor_add(ebias[:], nmax[:], b2s[:])
            ev = sb.tile([R, L], F32, tag="ev", name="ev")
            ssum = sb.tile([R, 1], F32, tag="ssum", name="ssum")
            nc.scalar.activation(
                ev[:], lms[:], ACT.Exp, bias=ebias[:, 0:1], accum_out=ssum[:]
            )
            rinv = sb.tile([R, 1], F32, tag="rinv", name="rinv")
            nc.vector.reciprocal(rinv[:], ssum[:])
            at = sb.tile([R, L], F32R, tag="at", name="at")
            nc.vector.tensor_scalar_mul(at[:], ev[:], rinv[:, 0:1])
            nc.sync.dma_start(attn_out[g, :, :], at[:])

            # ---- per segment: replicate attn, weight, graph-sum ----
            for k in range(gs):
                s = g * gs + k
                for j in range(2):
                    blk = (k * 2 + j) * 128
                    rep = ps.tile([128, L], F32, tag=f"rep{j}", name=f"rep{j}")
                    nc.tensor.matmul(
                        rep[:],
                        e4s[:, blk : blk + 128],
                        at[:],
                        start=True,
                        stop=True,
                    )
                    wsc = sb.tile([128, L], F32, tag=f"wsc{j}", name=f"wsc{j}")
                    gcol = (g0 if j == 0 else g1)[:, s : s + 1]
                    nc.vector.scalar_tensor_tensor(
                        wsc[:],
                        xts_all[k][j][:],
                        1.0,
                        rep[:],
                        op0=ALU.mult,
                        op1=ALU.mult,
                        accum_out=gcol,
                    )

        nc.sync.dma_start(g_out[0, :, :], g0[:])
        nc.sync.dma_start(g_out[1, :, :], g1[:])

    nc.finalize()
    return nc


def kernel(x, batch, W1, b1, W2, b2, Wt, bt):
    global LAST_EXEC_NS, LAST_RESULT

    x = np.ascontiguousarray(np.asarray(x, dtype=np.float32))
    batch = np.asarray(batch)
    N = x.shape[0]

    starts = np.searchsorted(batch, np.arange(B + 1)).astype(np.int64)
    lens = np.diff(starts)
    maxlen = int(lens.max())
    L = max(512, int(np.ceil(maxlen / 128.0)) * 128)

    if L not in _programs:
        _programs[L] = _build_program(L)
    nc = _programs[L]

    # node -> (segment, position-within-segment)
    pos = np.arange(N, dtype=np.int64) - starts[batch]

    mask1 = np.full((B, L), NEG_MASK, dtype=np.float32)
    mask1[batch, pos] = 0.0
    mask_all = np.ascontiguousarray(
        np.broadcast_to(mask1[:, None, :], (B, NUM_HEADS, L))
    )

    # padded, per-segment-transposed x: [B, H, L]
    xp = np.zeros((B, L, H), dtype=np.float32)
    xp[batch, pos] = x
    xpt_all = np.ascontiguousarray(xp.transpose(0, 2, 1))  # [B, H, L]
    del xp

    W1 = np.ascontiguousarray(np.asarray(W1, dtype=np.float32))
    W2 = np.ascontiguousarray(np.asarray(W2, dtype=np.float32))
    Wt = np.ascontiguousarray(np.asarray(Wt, dtype=np.float32))
    b1c = np.ascontiguousarray(np.asarray(b1, dtype=np.float32).reshape(H, 1))
    b2c = np.ascontiguousarray(
        np.tile(np.asarray(b2, dtype=np.float32).reshape(1, NUM_HEADS), (GS, 1)).reshape(
            GS * NUM_HEADS, 1
        )
    )
    e4 = np.zeros((GS, 2, GS * NUM_HEADS, 128), dtype=np.float32)
    for k in range(GS):
        for j in range(2):
            for hh in range(2):
                e4[k, j, 4 * k + 2 * j + hh, hh * HEAD_DIM : (hh + 1) * HEAD_DIM] = 1.0
    btc = np.ascontiguousarray(np.asarray(bt, dtype=np.float32).reshape(H, 1))

    in_maps = []
    for c in range(N_CORES):
        seg_lo = c * NSEG
        xpt_c = np.ascontiguousarray(
            xpt_all[seg_lo : seg_lo + NSEG].reshape(NSEG, 2, 128, L)
        )
        in_maps.append(
            dict(
                xpt=xpt_c,
                maskt=np.ascontiguousarray(mask_all[seg_lo : seg_lo + NSEG]),
                w1=W1,
                w2=W2,
                wt=Wt,
                b1d=b1c,
                b2d=b2c,
                e4d=e4,
                btd=btc,
            )
        )

    res = run_bass_kernel_spmd(nc, in_maps, list(range(N_CORES)), trace=TRACE)
    LAST_RESULT = res
    LAST_EXEC_NS = res.exec_time_ns

    # ---- unpack ----
    graph_emb = np.empty((B, H), dtype=np.float32)
    attn_all = np.empty((B, NUM_HEADS, L), dtype=np.float32)
    for c in range(N_CORES):
        seg_lo = c * NSEG
        g = res.results[c]["g_out"]  # [2, 128, NSEG]
        graph_emb[seg_lo : seg_lo + NSEG] = g.transpose(2, 0, 1).reshape(NSEG, H)
        attn_all[seg_lo : seg_lo + NSEG] = res.results[c]["attn_out"].reshape(
            NSEG, NUM_HEADS, L
        )

    graph_emb[lens == 0] = 0.0
    attn = attn_all[batch, :, pos]  # [N, 4]
    return graph_emb, attn


# revision 19
# speedup vs baseline: 1.3257x; 1.1054x over previous
"""AttentionReadout (segment softmax attention pooling) on 8 Trainium2 cores.

Math (reference):
    logits = tanh(x @ W1 + b1) @ W2 + b2          # [N, 4]
    attn   = segment_softmax(logits, batch)       # [N, 4]
    xt     = x @ Wt + bt                          # [N, 256] -> [N, 4, 64]
    graph_emb = segment_sum(attn[:, :, None] * xt)  # [1024, 256]
    returns (graph_emb, attn)

Strategy:
  * Host packs nodes into a segment-padded layout: every segment gets L
    (>=512, multiple of 128) node slots, padding rows are zero and carry a
    -30000 logit mask.  1024 segments / 8 cores = 128 whole segments per
    core -> segment reductions are device-local, fixed-shape free-dim
    reductions.
  * Host pre-transposes x per segment to [hidden, L] so every matmul uses
    the weights as natural lhsT ([K, M]) and the activations as rhs
    ([K, N=L]) -- no on-device transposes.
  * All matmuls run as float32r (full fp32 data, 1 cycle/row at N>=256).
  * exp+segment-sum fuse into one scalar-engine activation (accum_out);
    (xt + bt) * attn and the graph sum fuse into one DVE
    scalar_tensor_tensor with accum_out.
  * Outputs: attn in transposed padded layout [nseg, 4, L]; graph
    embeddings as [2, 128, nseg] columns.  Host scatters back.
"""

import numpy as np
from contextlib import ExitStack

import concourse.bass as bass
import concourse.bacc as bacc
import concourse.tile as tile
from concourse import mybir
from concourse.bass_utils import run_bass_kernel_spmd

N_CORES = 8
H = 256
NUM_HEADS = 4
HEAD_DIM = H // NUM_HEADS
B = 1024
NSEG = B // N_CORES  # segments per core
NEG_MASK = -30000.0
GS = 8  # segments per softmax batch group

F32 = mybir.dt.float32
F32R = mybir.dt.float32r
AX = mybir.AxisListType.X
ALU = mybir.AluOpType
ACT = mybir.ActivationFunctionType

# compiled program cache, keyed by L (pad length per segment)
_programs: dict[int, bass.Bass] = {}

LAST_EXEC_NS = None
LAST_RESULT = None
TRACE = False


def _build_program(L: int, nseg: int = NSEG, gs: int = GS) -> bass.Bass:
    ngroups = nseg // gs
    R = NUM_HEADS * gs  # stacked softmax rows per group
    nc = bacc.Bacc()

    xpt = nc.dram_tensor("xpt", [nseg, 2, 128, L], F32R, kind="ExternalInput")
    maskt = nc.dram_tensor("maskt", [nseg, NUM_HEADS, L], F32, kind="ExternalInput")
    w1 = nc.dram_tensor("w1", [H, H], F32R, kind="ExternalInput")
    w2 = nc.dram_tensor("w2", [H, NUM_HEADS], F32R, kind="ExternalInput")
    wt = nc.dram_tensor("wt", [H, H], F32R, kind="ExternalInput")
    b1d = nc.dram_tensor("b1d", [H, 1], F32, kind="ExternalInput")
    b2d = nc.dram_tensor("b2d", [R, 1], F32, kind="ExternalInput")
    btd = nc.dram_tensor("btd", [H, 1], F32, kind="ExternalInput")
    e4d = nc.dram_tensor("e4d", [gs, 2, R, 128], F32R, kind="ExternalInput")

    attn_out = nc.dram_tensor(
        "attn_out", [ngroups, R, L], F32R, kind="ExternalOutput"
    )
    g_out = nc.dram_tensor("g_out", [2, 128, nseg], F32, kind="ExternalOutput")

    with tile.TileContext(nc) as tc, ExitStack() as ctx:
        const = ctx.enter_context(tc.tile_pool(name="const", bufs=1))
        sb = ctx.enter_context(tc.tile_pool(name="sb", bufs=2))
        ps = ctx.enter_context(tc.tile_pool(name="ps", bufs=1, space="PSUM"))

        # --- persistent weights / constants ---
        w1s0 = const.tile([128, H], F32R, tag="w1s0", name="w1s0")
        w1s1 = const.tile([128, H], F32R, tag="w1s1", name="w1s1")
        nc.sync.dma_start(w1s0[:], w1[0:128, :])
        nc.sync.dma_start(w1s1[:], w1[128:256, :])
        wts0 = const.tile([128, H], F32R, tag="wts0", name="wts0")
        wts1 = const.tile([128, H], F32R, tag="wts1", name="wts1")
        nc.sync.dma_start(wts0[:], wt[0:128, :])
        nc.sync.dma_start(wts1[:], wt[128:256, :])
        w2s0 = const.tile([128, NUM_HEADS], F32R, tag="w2s0", name="w2s0")
        w2s1 = const.tile([128, NUM_HEADS], F32R, tag="w2s1", name="w2s1")
        nc.sync.dma_start(w2s0[:], w2[0:128, :])
        nc.sync.dma_start(w2s1[:], w2[128:256, :])
        b1s = const.tile([128, 2], F32, tag="b1s", name="b1s")
        nc.sync.dma_start(b1s[:, 0:1], b1d[0:128, :])
        nc.sync.dma_start(b1s[:, 1:2], b1d[128:256, :])
        bts = const.tile([128, 2], F32, tag="bts", name="bts")
        nc.sync.dma_start(bts[:, 0:1], btd[0:128, :])
        nc.sync.dma_start(bts[:, 1:2], btd[128:256, :])
        b2s = const.tile([R, 1], F32, tag="b2s", name="b2s")
        nc.sync.dma_start(b2s[:], b2d[:, :])
        e4s = const.tile([R, gs * 2 * 128], F32R, tag="e4s", name="e4s")
        for k in range(gs):
            for j in range(2):
                blk = (k * 2 + j) * 128
                nc.sync.dma_start(e4s[:, blk : blk + 128], e4d[k, j, :, :])

        # graph embedding accumulators: column s = segment s
        g0 = const.tile([128, nseg], F32, tag="g0", name="g0")
        g1 = const.tile([128, nseg], F32, tag="g1", name="g1")

        w1s = [w1s0, w1s1]
        wts = [wts0, wts1]
        w2s = [w2s0, w2s1]

        def phase2_seg(gp, kp, at_t, xts_list):
            sp = gp * gs + kp
            for j in range(2):
                blk = (kp * 2 + j) * 128
                rep = ps.tile([128, L], F32, tag=f"rep{j}", name=f"rep{j}")
                nc.tensor.matmul(
                    rep[:],
                    e4s[:, blk : blk + 128],
                    at_t[:],
                    start=True,
                    stop=True,
                )
                wsc = sb.tile([128, L], F32, tag=f"wsc{j}", name=f"wsc{j}")
                gcol = (g0 if j == 0 else g1)[:, sp : sp + 1]
                nc.vector.scalar_tensor_tensor(
                    wsc[:],
                    xts_list[kp][j][:],
                    1.0,
                    rep[:],
                    op0=ALU.mult,
                    op1=ALU.mult,
                    accum_out=gcol,
                )

        prev = None  # (g, at_tile, xts_all) of the previous group
        for g in range(ngroups):
            # masked logits of gs segments stacked on partitions:
            # row 4k+h = (seg k, head h)
            lms = sb.tile([R, L], F32, tag="lms", name="lms")

            xts_all = []
            for k in range(gs):
                s = g * gs + k
                xp0 = sb.tile([128, L], F32R, tag="xp0", bufs=3, name="xp0")
                xp1 = sb.tile([128, L], F32R, tag="xp1", bufs=3, name="xp1")
                nc.sync.dma_start(xp0[:], xpt[s, 0, :, :])
                nc.sync.dma_start(xp1[:], xpt[s, 1, :, :])
                xps = [xp0, xp1]

                # hT = tanh(W1^T x^T + b1)
                hs = []
                for j in range(2):
                    ph = ps.tile([128, L], F32, tag=f"ph{j}", name=f"ph{j}")
                    for i in range(2):
                        nc.tensor.matmul(
                            ph[:],
                            w1s[i][:, j * 128 : (j + 1) * 128],
                            xps[i][:],
                            start=(i == 0),
                            stop=(i == 1),
                        )
                    hj = sb.tile([128, L], F32R, tag=f"h{j}", name=f"h{j}")
                    nc.scalar.activation(
                        hj[:], ph[:], ACT.Tanh, bias=b1s[:, j : j + 1]
                    )
                    hs.append(hj)

                # logits -> psum [4, L]; DVE drain; gpsimd memset mask; stack
                pl = ps.tile([NUM_HEADS, L], F32, tag="pl", bufs=2, name="pl")
                for i in range(2):
                    nc.tensor.matmul(
                        pl[:], w2s[i][:], hs[i][:], start=(i == 0), stop=(i == 1)
                    )
                mk = sb.tile([NUM_HEADS, L], F32, tag="mk", bufs=3, name="mk")
                nc.sync.dma_start(mk[:], maskt[s, :, :])
                lml = sb.tile([NUM_HEADS, L], F32, tag="lml", bufs=3, name="lml")
                nc.vector.tensor_add(lml[:], pl[:], mk[:])
                nc.sync.dma_start(lms[4 * k : 4 * k + 4, :], lml[:])

                # xt^T = Wt^T x^T; drain psum->sbuf on ACT with bias fused
                xts_pair = []
                for j in range(2):
                    px = ps.tile([128, L], F32, tag=f"px{j}", name=f"px{j}")
                    for i in range(2):
                        nc.tensor.matmul(
                            px[:],
                            wts[i][:, j * 128 : (j + 1) * 128],
                            xps[i][:],
                            start=(i == 0),
                            stop=(i == 1),
                        )
                    xts = sb.tile(
                        [128, L], F32, tag=f"xts{j}", bufs=2 * gs + 2,
                        name=f"xts{j}",
                    )
                    nc.scalar.activation(
                        xts[:], px[:], ACT.Identity, bias=bts[:, j : j + 1]
                    )
                    xts_pair.append(xts)
                xts_all.append(xts_pair)
                if prev is not None:
                    phase2_seg(prev[0], k, prev[1], prev[2])

            # ---- batched segment softmax over the whole group ----
            nmax = sb.tile([R, 1], F32, tag="nmax", name="nmax")
            nc.vector.reduce_max(nmax[:], lms[:], axis=AX, negate=True)
            ebias = sb.tile([R, 1], F32, tag="ebias", name="ebias")
            nc.vector.tensor_add(ebias[:], nmax[:], b2s[:])
            ev = sb.tile([R, L], F32, tag="ev", name="ev")
            ssum = sb.tile([R, 1], F32, tag="ssum", name="ssum")
            nc.scalar.activation(
                ev[:], lms[:], ACT.Exp, bias=ebias[:, 0:1], accum_out=ssum[:]
            )
            rinv = sb.tile([R, 1], F32, tag="rinv", name="rinv")
            nc.vector.reciprocal(rinv[:], ssum[:])
            at = sb.tile([R, L], F32R, tag="at", name="at")
            nc.vector.tensor_scalar_mul(at[:], ev[:], rinv[:, 0:1])
            nc.sync.dma_start(attn_out[g, :, :], at[:])

            prev = (g, at, xts_all)

        # drain the last group's phase 2
        for k in range(gs):
            phase2_seg(prev[0], k, prev[1], prev[2])

        nc.sync.dma_start(g_out[0, :, :], g0[:])
        nc.sync.dma_start(g_out[1, :, :], g1[:])

    nc.finalize()
    return nc


def kernel(x, batch, W1, b1, W2, b2, Wt, bt):
    global LAST_EXEC_NS, LAST_RESULT

    x = np.ascontiguousarray(np.asarray(x, dtype=np.float32))
    batch = np.asarray(batch)
    N = x.shape[0]

    starts = np.searchsorted(batch, np.arange(B + 1)).astype(np.int64)
    lens = np.diff(starts)
    maxlen = int(lens.max())
    L = max(512, int(np.ceil(maxlen / 128.0)) * 128)

    if L not in _programs:
        _programs[L] = _build_program(L)
    nc = _programs[L]

    # node -> (segment, position-within-segment)
    pos = np.arange(N, dtype=np.int64) - starts[batch]

    mask1 = np.full((B, L), NEG_MASK, dtype=np.float32)
    mask1[batch, pos] = 0.0
    mask_all = np.ascontiguousarray(
        np.broadcast_to(mask1[:, None, :], (B, NUM_HEADS, L))
    )

    # padded, per-segment-transposed x: [B, H, L]
    xp = np.zeros((B, L, H), dtype=np.float32)
    xp[batch, pos] = x
    xpt_all = np.ascontiguousarray(xp.transpose(0, 2, 1))  # [B, H, L]
    del xp

    W1 = np.ascontiguousarray(np.asarray(W1, dtype=np.float32))
    W2 = np.ascontiguousarray(np.asarray(W2, dtype=np.float32))
    Wt = np.ascontiguousarray(np.asarray(Wt, dtype=np.float32))
    b1c = np.ascontiguousarray(np.asarray(b1, dtype=np.float32).reshape(H, 1))
    b2c = np.ascontiguousarray(
        np.tile(np.asarray(b2, dtype=np.float32).reshape(1, NUM_HEADS), (GS, 1)).reshape(
            GS * NUM_HEADS, 1
        )
    )
    e4 = np.zeros((GS, 2, GS * NUM_HEADS, 128), dtype=np.float32)
    for k in range(GS):
        for j in range(2):
            for hh in range(2):
                e4[k, j, 4 * k + 2 * j + hh, hh * HEAD_DIM : (hh + 1) * HEAD_DIM] = 1.0
    btc = np.ascontiguousarray(np.asarray(bt, dtype=np.float32).reshape(H, 1))

    in_maps = []
    for c in range(N_CORES):
        seg_lo = c * NSEG
        xpt_c = np.ascontiguousarray(
            xpt_all[seg_lo : seg_lo + NSEG].reshape(NSEG, 2, 128, L)
        )
        in_maps.append(
            dict(
                xpt=xpt_c,
                maskt=np.ascontiguousarray(mask_all[seg_lo : seg_lo + NSEG]),
                w1=W1,
                w2=W2,
                wt=Wt,
                b1d=b1c,
                b2d=b2c,
                e4d=e4,
                btd=btc,
            )
        )

    res = run_bass_kernel_spmd(nc, in_maps, list(range(N_CORES)), trace=TRACE)
    LAST_RESULT = res
    LAST_EXEC_NS = res.exec_time_ns

    # ---- unpack ----
    graph_emb = np.empty((B, H), dtype=np.float32)
    attn_all = np.empty((B, NUM_HEADS, L), dtype=np.float32)
    for c in range(N_CORES):
        seg_lo = c * NSEG
        g = res.results[c]["g_out"]  # [2, 128, NSEG]
        graph_emb[seg_lo : seg_lo + NSEG] = g.transpose(2, 0, 1).reshape(NSEG, H)
        attn_all[seg_lo : seg_lo + NSEG] = res.results[c]["attn_out"].reshape(
            NSEG, NUM_HEADS, L
        )

    graph_emb[lens == 0] = 0.0
    attn = attn_all[batch, :, pos]  # [N, 4]
    return graph_emb, attn


# revision 20
# speedup vs baseline: 1.5517x; 1.1704x over previous
"""AttentionReadout (segment softmax attention pooling) on 8 Trainium2 cores.

Math (reference):
    logits = tanh(x @ W1 + b1) @ W2 + b2          # [N, 4]
    attn   = segment_softmax(logits, batch)       # [N, 4]
    xt     = x @ Wt + bt                          # [N, 256] -> [N, 4, 64]
    graph_emb = segment_sum(attn[:, :, None] * xt)  # [1024, 256]
    returns (graph_emb, attn)

Strategy:
  * Host packs nodes into a segment-padded layout: every segment gets L
    (>=512, multiple of 128) node slots, padding rows are zero and carry a
    -30000 logit mask.  1024 segments / 8 cores = 128 whole segments per
    core -> segment reductions are device-local, fixed-shape free-dim
    reductions.
  * Host pre-transposes x per segment to [hidden, L] so every matmul uses
    the weights as natural lhsT ([K, M]) and the activations as rhs
    ([K, N=L]) -- no on-device transposes.
  * All matmuls run as float32r (full fp32 data, 1 cycle/row at N>=256).
  * exp+segment-sum fuse into one scalar-engine activation (accum_out);
    (xt + bt) * attn and the graph sum fuse into one DVE
    scalar_tensor_tensor with accum_out.
  * Outputs: attn in transposed padded layout [nseg, 4, L]; graph
    embeddings as [2, 128, nseg] columns.  Host scatters back.
"""

import numpy as np
from contextlib import ExitStack

import concourse.bass as bass
import concourse.bacc as bacc
import concourse.tile as tile
from concourse import mybir
from concourse.bass_utils import run_bass_kernel_spmd

N_CORES = 8
H = 256
NUM_HEADS = 4
HEAD_DIM = H // NUM_HEADS
B = 1024
NSEG = B // N_CORES  # segments per core
NEG_MASK = -30000.0
GS = 8  # segments per softmax batch group

F32 = mybir.dt.float32
F32R = mybir.dt.float32r
AX = mybir.AxisListType.X
ALU = mybir.AluOpType
ACT = mybir.ActivationFunctionType

# compiled program cache, keyed by L (pad length per segment)
_programs: dict[int, bass.Bass] = {}

LAST_EXEC_NS = None
LAST_RESULT = None
TRACE = False


def _build_program(L: int, nseg: int = NSEG, gs: int = GS) -> bass.Bass:
    ngroups = nseg // gs
    R = NUM_HEADS * gs  # stacked softmax rows per group
    nc = bacc.Bacc()

    xpt = nc.dram_tensor("xpt", [nseg, 128, 2 * L], F32R, kind="ExternalInput")
    maskt = nc.dram_tensor("maskt", [ngroups, NUM_HEADS, gs * L], F32, kind="ExternalInput")
    w1 = nc.dram_tensor("w1", [H, H], F32R, kind="ExternalInput")
    w2 = nc.dram_tensor("w2", [H, NUM_HEADS], F32R, kind="ExternalInput")
    wt = nc.dram_tensor("wt", [H, H], F32R, kind="ExternalInput")
    b1d = nc.dram_tensor("b1d", [H, 1], F32, kind="ExternalInput")
    b2d = nc.dram_tensor("b2d", [R, 1], F32, kind="ExternalInput")
    btd = nc.dram_tensor("btd", [H, 1], F32, kind="ExternalInput")
    e4d = nc.dram_tensor("e4d", [gs, 2, R, 128], F32R, kind="ExternalInput")

    attn_out = nc.dram_tensor(
        "attn_out", [ngroups, R, L], F32R, kind="ExternalOutput"
    )
    g_out = nc.dram_tensor("g_out", [2, 128, nseg], F32, kind="ExternalOutput")

    with tile.TileContext(nc) as tc, ExitStack() as ctx:
        const = ctx.enter_context(tc.tile_pool(name="const", bufs=1))
        sb = ctx.enter_context(tc.tile_pool(name="sb", bufs=2))
        ps = ctx.enter_context(tc.tile_pool(name="ps", bufs=1, space="PSUM"))

        # --- persistent weights / constants ---
        w1s0 = const.tile([128, H], F32R, tag="w1s0", name="w1s0")
        w1s1 = const.tile([128, H], F32R, tag="w1s1", name="w1s1")
        nc.sync.dma_start(w1s0[:], w1[0:128, :])
        nc.sync.dma_start(w1s1[:], w1[128:256, :])
        wts0 = const.tile([128, H], F32R, tag="wts0", name="wts0")
        wts1 = const.tile([128, H], F32R, tag="wts1", name="wts1")
        nc.sync.dma_start(wts0[:], wt[0:128, :])
        nc.sync.dma_start(wts1[:], wt[128:256, :])
        w2s0 = const.tile([128, NUM_HEADS], F32R, tag="w2s0", name="w2s0")
        w2s1 = const.tile([128, NUM_HEADS], F32R, tag="w2s1", name="w2s1")
        nc.sync.dma_start(w2s0[:], w2[0:128, :])
        nc.sync.dma_start(w2s1[:], w2[128:256, :])
        b1s = const.tile([128, 2], F32, tag="b1s", name="b1s")
        nc.sync.dma_start(b1s[:, 0:1], b1d[0:128, :])
        nc.sync.dma_start(b1s[:, 1:2], b1d[128:256, :])
        bts = const.tile([128, 2], F32, tag="bts", name="bts")
        nc.sync.dma_start(bts[:, 0:1], btd[0:128, :])
        nc.sync.dma_start(bts[:, 1:2], btd[128:256, :])
        b2s = const.tile([R, 1], F32, tag="b2s", name="b2s")
        nc.sync.dma_start(b2s[:], b2d[:, :])
        e4s = const.tile([R, gs * 2 * 128], F32R, tag="e4s", name="e4s")
        for k in range(gs):
            for j in range(2):
                blk = (k * 2 + j) * 128
                nc.sync.dma_start(e4s[:, blk : blk + 128], e4d[k, j, :, :])

        # graph embedding accumulators: column s = segment s
        g0 = const.tile([128, nseg], F32, tag="g0", name="g0")
        g1 = const.tile([128, nseg], F32, tag="g1", name="g1")

        w1s = [w1s0, w1s1]
        wts = [wts0, wts1]
        w2s = [w2s0, w2s1]

        def phase2_seg(gp, kp, at_t, xts_list):
            sp = gp * gs + kp
            for j in range(2):
                blk = (kp * 2 + j) * 128
                rep = ps.tile([128, L], F32, tag="rep", bufs=3, name="rep")
                nc.tensor.matmul(
                    rep[:],
                    e4s[:, blk : blk + 128],
                    at_t[:],
                    start=True,
                    stop=True,
                )
                wsc = sb.tile([128, L], F32, tag=f"wsc{j}", name=f"wsc{j}")
                gcol = (g0 if j == 0 else g1)[:, sp : sp + 1]
                nc.vector.scalar_tensor_tensor(
                    wsc[:],
                    xts_list[kp][j][:],
                    1.0,
                    rep[:],
                    op0=ALU.mult,
                    op1=ALU.mult,
                    accum_out=gcol,
                )

        prev = None  # (g, at_tile, xts_all) of the previous group
        for g in range(ngroups):
            # masked logits of gs segments stacked on partitions:
            # row 4k+h = (seg k, head h)
            lms = sb.tile([R, L], F32, tag="lms", name="lms")
            mkg = sb.tile([NUM_HEADS, gs * L], F32, tag="mkg", name="mkg")
            nc.sync.dma_start(mkg[:], maskt[g, :, :])

            xts_all = []
            for k in range(gs):
                s = g * gs + k
                xp = sb.tile([128, 2 * L], F32R, tag="xp", bufs=3, name="xp")
                nc.sync.dma_start(xp[:], xpt[s, :, :])
                xps = [xp[:, 0:L], xp[:, L : 2 * L]]

                # hT = tanh(W1^T x^T + b1)
                hs = []
                for j in range(2):
                    ph = ps.tile([128, L], F32, tag=f"ph{j}", name=f"ph{j}")
                    for i in range(2):
                        nc.tensor.matmul(
                            ph[:],
                            w1s[i][:, j * 128 : (j + 1) * 128],
                            xps[i],
                            start=(i == 0),
                            stop=(i == 1),
                        )
                    hj = sb.tile([128, L], F32R, tag=f"h{j}", name=f"h{j}")
                    nc.scalar.activation(
                        hj[:], ph[:], ACT.Tanh, bias=b1s[:, j : j + 1]
                    )
                    hs.append(hj)

                # xt^T = Wt^T x^T; drain psum->sbuf on ACT with bias fused
                xts_pair = []
                for j in range(2):
                    px = ps.tile([128, L], F32, tag=f"px{j}", name=f"px{j}")
                    for i in range(2):
                        nc.tensor.matmul(
                            px[:],
                            wts[i][:, j * 128 : (j + 1) * 128],
                            xps[i],
                            start=(i == 0),
                            stop=(i == 1),
                        )
                    xts = sb.tile(
                        [128, L], F32, tag=f"xts{j}", bufs=2 * gs + 2,
                        name=f"xts{j}",
                    )
                    nc.scalar.activation(
                        xts[:], px[:], ACT.Identity, bias=bts[:, j : j + 1]
                    )
                    xts_pair.append(xts)
                xts_all.append(xts_pair)

                # logits -> psum [4, L]; DVE drain+mask; DMA-stack
                pl = ps.tile([NUM_HEADS, L], F32, tag="pl", bufs=1, name="pl")
                for i in range(2):
                    nc.tensor.matmul(
                        pl[:], w2s[i][:], hs[i][:], start=(i == 0), stop=(i == 1)
                    )
                lml = sb.tile([NUM_HEADS, L], F32, tag="lml", bufs=3, name="lml")
                nc.vector.tensor_add(lml[:], pl[:], mkg[:, k * L : (k + 1) * L])
                nc.sync.dma_start(lms[4 * k : 4 * k + 4, :], lml[:])
                if prev is not None:
                    phase2_seg(prev[0], k, prev[1], prev[2])

            # ---- batched segment softmax over the whole group ----
            nmax = sb.tile([R, 1], F32, tag="nmax", name="nmax")
            nc.vector.reduce_max(nmax[:], lms[:], axis=AX, negate=True)
            ebias = sb.tile([R, 1], F32, tag="ebias", name="ebias")
            nc.vector.tensor_add(ebias[:], nmax[:], b2s[:])
            ev = sb.tile([R, L], F32, tag="ev", name="ev")
            ssum = sb.tile([R, 1], F32, tag="ssum", name="ssum")
            nc.scalar.activation(
                ev[:], lms[:], ACT.Exp, bias=ebias[:, 0:1], accum_out=ssum[:]
            )
            rinv = sb.tile([R, 1], F32, tag="rinv", name="rinv")
            nc.vector.reciprocal(rinv[:], ssum[:])
            at = sb.tile([R, L], F32R, tag="at", name="at")
            nc.vector.tensor_scalar_mul(at[:], ev[:], rinv[:, 0:1])
            nc.sync.dma_start(attn_out[g, :, :], at[:])

            prev = (g, at, xts_all)

        # drain the last group's phase 2
        for k in range(gs):
            phase2_seg(prev[0], k, prev[1], prev[2])

        nc.sync.dma_start(g_out[0, :, :], g0[:])
        nc.sync.dma_start(g_out[1, :, :], g1[:])

    nc.finalize()
    return nc


def kernel(x, batch, W1, b1, W2, b2, Wt, bt):
    global LAST_EXEC_NS, LAST_RESULT

    x = np.ascontiguousarray(np.asarray(x, dtype=np.float32))
    batch = np.asarray(batch)
    N = x.shape[0]

    starts = np.searchsorted(batch, np.arange(B + 1)).astype(np.int64)
    lens = np.diff(starts)
    maxlen = int(lens.max())
    L = max(512, int(np.ceil(maxlen / 128.0)) * 128)

    if L not in _programs:
        _programs[L] = _build_program(L)
    nc = _programs[L]

    # node -> (segment, position-within-segment)
    pos = np.arange(N, dtype=np.int64) - starts[batch]

    mask1 = np.full((B, L), NEG_MASK, dtype=np.float32)
    mask1[batch, pos] = 0.0
    mask_all = np.ascontiguousarray(
        np.broadcast_to(mask1[:, None, :], (B, NUM_HEADS, L))
    )

    # padded, per-segment-transposed x: [B, H, L]
    xp = np.zeros((B, L, H), dtype=np.float32)
    xp[batch, pos] = x
    xpt_all = np.ascontiguousarray(xp.transpose(0, 2, 1))  # [B, H, L]
    del xp

    W1 = np.ascontiguousarray(np.asarray(W1, dtype=np.float32))
    W2 = np.ascontiguousarray(np.asarray(W2, dtype=np.float32))
    Wt = np.ascontiguousarray(np.asarray(Wt, dtype=np.float32))
    b1c = np.ascontiguousarray(np.asarray(b1, dtype=np.float32).reshape(H, 1))
    b2c = np.ascontiguousarray(
        np.tile(np.asarray(b2, dtype=np.float32).reshape(1, NUM_HEADS), (GS, 1)).reshape(
            GS * NUM_HEADS, 1
        )
    )
    e4 = np.zeros((GS, 2, GS * NUM_HEADS, 128), dtype=np.float32)
    for k in range(GS):
        for j in range(2):
            for hh in range(2):
                e4[k, j, 4 * k + 2 * j + hh, hh * HEAD_DIM : (hh + 1) * HEAD_DIM] = 1.0
    btc = np.ascontiguousarray(np.asarray(bt, dtype=np.float32).reshape(H, 1))

    in_maps = []
    for c in range(N_CORES):
        seg_lo = c * NSEG
        xpt_c = np.ascontiguousarray(
            xpt_all[seg_lo : seg_lo + NSEG]
            .reshape(NSEG, 2, 128, L)
            .transpose(0, 2, 1, 3)
            .reshape(NSEG, 128, 2 * L)
        )
        in_maps.append(
            dict(
                xpt=xpt_c,
                maskt=np.ascontiguousarray(
                    mask_all[seg_lo : seg_lo + NSEG]
                    .reshape(NSEG // GS, GS, NUM_HEADS, L)
                    .transpose(0, 2, 1, 3)
                    .reshape(NSEG // GS, NUM_HEADS, GS * L)
                ),
                w1=W1,
                w2=W2,
                wt=Wt,
                b1d=b1c,
                b2d=b2c,
                e4d=e4,
                btd=btc,
            )
        )

    res = run_bass_kernel_spmd(nc, in_maps, list(range(N_CORES)), trace=TRACE)
    LAST_RESULT = res
    LAST_EXEC_NS = res.exec_time_ns

    # ---- unpack ----
    graph_emb = np.empty((B, H), dtype=np.float32)
    attn_all = np.empty((B, NUM_HEADS, L), dtype=np.float32)
    for c in range(N_CORES):
        seg_lo = c * NSEG
        g = res.results[c]["g_out"]  # [2, 128, NSEG]
        graph_emb[seg_lo : seg_lo + NSEG] = g.transpose(2, 0, 1).reshape(NSEG, H)
        attn_all[seg_lo : seg_lo + NSEG] = res.results[c]["attn_out"].reshape(
            NSEG, NUM_HEADS, L
        )

    graph_emb[lens == 0] = 0.0
    attn = attn_all[batch, :, pos]  # [N, 4]
    return graph_emb, attn


# revision 21
# speedup vs baseline: 1.7215x; 1.1094x over previous
"""AttentionReadout (segment softmax attention pooling) on 8 Trainium2 cores.

Math (reference):
    logits = tanh(x @ W1 + b1) @ W2 + b2          # [N, 4]
    attn   = segment_softmax(logits, batch)       # [N, 4]
    xt     = x @ Wt + bt                          # [N, 256] -> [N, 4, 64]
    graph_emb = segment_sum(attn[:, :, None] * xt)  # [1024, 256]
    returns (graph_emb, attn)

Strategy:
  * Host packs nodes into a segment-padded layout: every segment gets L
    (>=512, multiple of 128) node slots, padding rows are zero and carry a
    -30000 logit mask.  1024 segments / 8 cores = 128 whole segments per
    core -> segment reductions are device-local, fixed-shape free-dim
    reductions.
  * Host pre-transposes x per segment to [hidden, L] so every matmul uses
    the weights as natural lhsT ([K, M]) and the activations as rhs
    ([K, N=L]) -- no on-device transposes.
  * All matmuls run as float32r (full fp32 data, 1 cycle/row at N>=256).
  * exp+segment-sum fuse into one scalar-engine activation (accum_out);
    (xt + bt) * attn and the graph sum fuse into one DVE
    scalar_tensor_tensor with accum_out.
  * Outputs: attn in transposed padded layout [nseg, 4, L]; graph
    embeddings as [2, 128, nseg] columns.  Host scatters back.
"""

import numpy as np
from contextlib import ExitStack

import concourse.bass as bass
import concourse.bacc as bacc
import concourse.tile as tile
from concourse import mybir
from concourse.bass_utils import run_bass_kernel_spmd

N_CORES = 8
H = 256
NUM_HEADS = 4
HEAD_DIM = H // NUM_HEADS
B = 1024
NSEG = B // N_CORES  # segments per core
NEG_MASK = -30000.0
GS = 8  # segments per softmax batch group

F32 = mybir.dt.float32
F32R = mybir.dt.float32r
AX = mybir.AxisListType.X
ALU = mybir.AluOpType
ACT = mybir.ActivationFunctionType

# compiled program cache, keyed by L (pad length per segment)
_programs: dict[int, bass.Bass] = {}

LAST_EXEC_NS = None
LAST_RESULT = None
TRACE = False


def _build_program(Ls, nseg: int = NSEG, gs: int = GS) -> bass.Bass:
    """Ls: per-group slot widths (len ngroups); group g holds gs segments,
    each padded to Ls[g] node slots."""
    ngroups = nseg // gs
    R = NUM_HEADS * gs
    Wx = int(sum(2 * Lg for Lg in Ls) * gs)  # xpt free width per partition
    Wm = int(sum(gs * Lg for Lg in Ls))  # mask free width
    Wa = int(sum(Lg for Lg in Ls))  # attn-out free width per group-row
    nc = bacc.Bacc()

    xpt = nc.dram_tensor("xpt", [128, Wx], F32R, kind="ExternalInput")
    maskt = nc.dram_tensor("maskt", [NUM_HEADS, Wm], F32, kind="ExternalInput")
    w1 = nc.dram_tensor("w1", [H, H], F32R, kind="ExternalInput")
    w2 = nc.dram_tensor("w2", [H, NUM_HEADS], F32R, kind="ExternalInput")
    wt = nc.dram_tensor("wt", [H, H], F32R, kind="ExternalInput")
    b1d = nc.dram_tensor("b1d", [H, 1], F32, kind="ExternalInput")
    b2d = nc.dram_tensor("b2d", [R, 1], F32, kind="ExternalInput")
    btd = nc.dram_tensor("btd", [H, 1], F32, kind="ExternalInput")
    e4d = nc.dram_tensor("e4d", [gs, 2, R, 128], F32R, kind="ExternalInput")

    attn_out = nc.dram_tensor("attn_out", [R, Wa], F32R, kind="ExternalOutput")
    g_out = nc.dram_tensor("g_out", [2, 128, nseg], F32, kind="ExternalOutput")

    Lmax = max(Ls)

    with tile.TileContext(nc) as tc, ExitStack() as ctx:
        const = ctx.enter_context(tc.tile_pool(name="const", bufs=1))
        sb = ctx.enter_context(tc.tile_pool(name="sb", bufs=2))
        ps = ctx.enter_context(tc.tile_pool(name="ps", bufs=1, space="PSUM"))

        # --- persistent weights / constants ---
        w1s0 = const.tile([128, H], F32R, tag="w1s0", name="w1s0")
        w1s1 = const.tile([128, H], F32R, tag="w1s1", name="w1s1")
        nc.sync.dma_start(w1s0[:], w1[0:128, :])
        nc.sync.dma_start(w1s1[:], w1[128:256, :])
        wts0 = const.tile([128, H], F32R, tag="wts0", name="wts0")
        wts1 = const.tile([128, H], F32R, tag="wts1", name="wts1")
        nc.sync.dma_start(wts0[:], wt[0:128, :])
        nc.sync.dma_start(wts1[:], wt[128:256, :])
        w2s0 = const.tile([128, NUM_HEADS], F32R, tag="w2s0", name="w2s0")
        w2s1 = const.tile([128, NUM_HEADS], F32R, tag="w2s1", name="w2s1")
        nc.sync.dma_start(w2s0[:], w2[0:128, :])
        nc.sync.dma_start(w2s1[:], w2[128:256, :])
        b1s = const.tile([128, 2], F32, tag="b1s", name="b1s")
        nc.sync.dma_start(b1s[:, 0:1], b1d[0:128, :])
        nc.sync.dma_start(b1s[:, 1:2], b1d[128:256, :])
        bts = const.tile([128, 2], F32, tag="bts", name="bts")
        nc.sync.dma_start(bts[:, 0:1], btd[0:128, :])
        nc.sync.dma_start(bts[:, 1:2], btd[128:256, :])
        b2s = const.tile([R, 1], F32, tag="b2s", name="b2s")
        nc.sync.dma_start(b2s[:], b2d[:, :])
        e4s = const.tile([R, gs * 2 * 128], F32R, tag="e4s", name="e4s")
        for k in range(gs):
            for j in range(2):
                blk = (k * 2 + j) * 128
                nc.sync.dma_start(e4s[:, blk : blk + 128], e4d[k, j, :, :])

        g0 = const.tile([128, nseg], F32, tag="g0", name="g0")
        g1 = const.tile([128, nseg], F32, tag="g1", name="g1")

        w1s = [w1s0, w1s1]
        wts = [wts0, wts1]
        w2s = [w2s0, w2s1]

        def phase2_seg(gp, kp, at_t, xts_list, Lg):
            sp = gp * gs + kp
            for j in range(2):
                blk = (kp * 2 + j) * 128
                rep = ps.tile([128, Lmax], F32, tag="rep", bufs=3, name="rep")
                nc.tensor.matmul(
                    rep[:, 0:Lg],
                    e4s[:, blk : blk + 128],
                    at_t[:, 0:Lg],
                    start=True,
                    stop=True,
                )
                wsc = sb.tile([128, Lmax], F32, tag=f"wsc{j}", name=f"wsc{j}")
                gcol = (g0 if j == 0 else g1)[:, sp : sp + 1]
                nc.vector.scalar_tensor_tensor(
                    wsc[:, 0:Lg],
                    xts_list[kp][j][:, 0:Lg],
                    1.0,
                    rep[:, 0:Lg],
                    op0=ALU.mult,
                    op1=ALU.mult,
                    accum_out=gcol,
                )

        xoff = 0  # running offset into xpt free dim
        moff = 0  # running offset into maskt free dim
        aoff = 0  # running offset into attn_out free dim
        prev = None
        for g in range(ngroups):
            Lg = int(Ls[g])
            lms = sb.tile([R, Lmax], F32, tag="lms", name="lms")
            mkg = sb.tile([NUM_HEADS, gs * Lmax], F32, tag="mkg", name="mkg")
            nc.sync.dma_start(
                mkg[:, 0 : gs * Lg], maskt[:, moff : moff + gs * Lg]
            )
            moff += gs * Lg

            xts_all = []
            for k in range(gs):
                s = g * gs + k
                xp = sb.tile([128, 2 * Lmax], F32R, tag="xp", bufs=3, name="xp")
                nc.sync.dma_start(
                    xp[:, 0 : 2 * Lg], xpt[:, xoff : xoff + 2 * Lg]
                )
                xoff += 2 * Lg
                xps = [xp[:, 0:Lg], xp[:, Lg : 2 * Lg]]

                hs = []
                for j in range(2):
                    ph = ps.tile([128, Lmax], F32, tag=f"ph{j}", name=f"ph{j}")
                    for i in range(2):
                        nc.tensor.matmul(
                            ph[:, 0:Lg],
                            w1s[i][:, j * 128 : (j + 1) * 128],
                            xps[i],
                            start=(i == 0),
                            stop=(i == 1),
                        )
                    hj = sb.tile([128, Lmax], F32R, tag=f"h{j}", name=f"h{j}")
                    nc.scalar.activation(
                        hj[:, 0:Lg], ph[:, 0:Lg], ACT.Tanh, bias=b1s[:, j : j + 1]
                    )
                    hs.append(hj)

                xts_pair = []
                for j in range(2):
                    px = ps.tile([128, Lmax], F32, tag=f"px{j}", name=f"px{j}")
                    for i in range(2):
                        nc.tensor.matmul(
                            px[:, 0:Lg],
                            wts[i][:, j * 128 : (j + 1) * 128],
                            xps[i],
                            start=(i == 0),
                            stop=(i == 1),
                        )
                    xts = sb.tile(
                        [128, Lmax], F32, tag=f"xts{j}", bufs=2 * gs + 2,
                        name=f"xts{j}",
                    )
                    nc.scalar.activation(
                        xts[:, 0:Lg], px[:, 0:Lg], ACT.Identity,
                        bias=bts[:, j : j + 1],
                    )
                    xts_pair.append(xts)
                xts_all.append(xts_pair)

                pl = ps.tile([NUM_HEADS, Lmax], F32, tag="pl", bufs=1, name="pl")
                for i in range(2):
                    nc.tensor.matmul(
                        pl[:, 0:Lg],
                        w2s[i][:],
                        hs[i][:, 0:Lg],
                        start=(i == 0),
                        stop=(i == 1),
                    )
                lml = sb.tile([NUM_HEADS, Lmax], F32, tag="lml", bufs=3, name="lml")
                nc.vector.tensor_add(
                    lml[:, 0:Lg], pl[:, 0:Lg], mkg[:, k * Lg : (k + 1) * Lg]
                )
                nc.sync.dma_start(lms[4 * k : 4 * k + 4, 0:Lg], lml[:, 0:Lg])

                if prev is not None:
                    phase2_seg(prev[0], k, prev[1], prev[2], prev[3])

            # ---- batched segment softmax over the whole group ----
            nmax = sb.tile([R, 1], F32, tag="nmax", name="nmax")
            nc.vector.reduce_max(nmax[:], lms[:, 0:Lg], axis=AX, negate=True)
            ebias = sb.tile([R, 1], F32, tag="ebias", name="ebias")
            nc.vector.tensor_add(ebias[:], nmax[:], b2s[:])
            ev = sb.tile([R, Lmax], F32, tag="ev", name="ev")
            ssum = sb.tile([R, 1], F32, tag="ssum", name="ssum")
            nc.scalar.activation(
                ev[:, 0:Lg], lms[:, 0:Lg], ACT.Exp, bias=ebias[:, 0:1],
                accum_out=ssum[:],
            )
            rinv = sb.tile([R, 1], F32, tag="rinv", name="rinv")
            nc.vector.reciprocal(rinv[:], ssum[:])
            at = sb.tile([R, Lmax], F32R, tag="at", name="at")
            nc.vector.tensor_scalar_mul(at[:, 0:Lg], ev[:, 0:Lg], rinv[:, 0:1])
            nc.sync.dma_start(attn_out[:, aoff : aoff + Lg], at[:, 0:Lg])
            aoff += Lg

            prev = (g, at, xts_all, Lg)

        for k in range(gs):
            phase2_seg(prev[0], k, prev[1], prev[2], prev[3])

        nc.sync.dma_start(g_out[0, :, :], g0[:])
        nc.sync.dma_start(g_out[1, :, :], g1[:])

    nc.finalize()
    return nc


def kernel(x, batch, W1, b1, W2, b2, Wt, bt):
    global LAST_EXEC_NS, LAST_RESULT

    x = np.ascontiguousarray(np.asarray(x, dtype=np.float32))
    batch = np.asarray(batch)
    N = x.shape[0]

    starts = np.searchsorted(batch, np.arange(B + 1)).astype(np.int64)
    lens = np.diff(starts)
    maxlen = int(lens.max())
    assert maxlen <= 512, f"segment length {maxlen} > 512 unsupported"

    # global sort by length; rank r -> (core r%8, slot r//8); group = slot//GS
    seg_perm = np.argsort(lens, kind="stable")  # seg ids in ascending length
    ngroups = NSEG // GS
    # group widths: max len among ranks in the group's 8*GS-range, >=256, mult of 16
    Ls = []
    for gidx in range(ngroups):
        hi = lens[seg_perm[(gidx + 1) * N_CORES * GS - 1]]
        Lg = max(256, int(np.ceil(hi / 16.0)) * 16)
        Ls.append(Lg)
    key = tuple(Ls)
    if key not in _programs:
        _programs.clear()
        _programs[key] = _build_program(Ls)
    nc = _programs[key]

    # node -> (segment, position-within-segment)
    pos = np.arange(N, dtype=np.int64) - starts[batch]

    # per (core, slot): seg id
    slot_seg = seg_perm.reshape(NSEG, N_CORES)  # [slot, core]

    W1 = np.ascontiguousarray(np.asarray(W1, dtype=np.float32))
    W2 = np.ascontiguousarray(np.asarray(W2, dtype=np.float32))
    Wt = np.ascontiguousarray(np.asarray(Wt, dtype=np.float32))
    b1c = np.ascontiguousarray(np.asarray(b1, dtype=np.float32).reshape(H, 1))
    b2c = np.ascontiguousarray(
        np.tile(np.asarray(b2, dtype=np.float32).reshape(1, NUM_HEADS), (GS, 1)).reshape(
            GS * NUM_HEADS, 1
        )
    )
    btc = np.ascontiguousarray(np.asarray(bt, dtype=np.float32).reshape(H, 1))
    e4 = np.zeros((GS, 2, GS * NUM_HEADS, 128), dtype=np.float32)
    for k in range(GS):
        for j in range(2):
            for hh in range(2):
                e4[k, j, 4 * k + 2 * j + hh, hh * HEAD_DIM : (hh + 1) * HEAD_DIM] = 1.0

    Wx = sum(2 * Lg for Lg in Ls) * GS
    Wm = sum(GS * Lg for Lg in Ls)
    Wa = sum(Lg for Lg in Ls)

    # pack per core
    xpt_cores = [np.zeros((128, Wx), np.float32) for _ in range(N_CORES)]
    mask_cores = [np.full((NUM_HEADS, Wm), NEG_MASK, np.float32) for _ in range(N_CORES)]
    xT = np.ascontiguousarray(x.T)  # [H, N]
    for c in range(N_CORES):
        xoff = 0
        moff = 0
        xc = xpt_cores[c]
        mc = mask_cores[c]
        for gidx in range(ngroups):
            Lg = Ls[gidx]
            for k in range(GS):
                slot = gidx * GS + k
                seg = slot_seg[slot, c]
                ln = int(lens[seg])
                s0 = int(starts[seg])
                if ln:
                    xc[:, xoff : xoff + ln] = xT[0:128, s0 : s0 + ln]
                    xc[:, xoff + Lg : xoff + Lg + ln] = xT[128:256, s0 : s0 + ln]
                    mc[:, moff + k * Lg : moff + k * Lg + ln] = 0.0
                xoff += 2 * Lg
            moff += GS * Lg

    in_maps = []
    for c in range(N_CORES):
        in_maps.append(
            dict(
                xpt=xpt_cores[c],
                maskt=mask_cores[c],
                w1=W1,
                w2=W2,
                wt=Wt,
                b1d=b1c,
                b2d=b2c,
                btd=btc,
                e4d=e4,
            )
        )

    res = run_bass_kernel_spmd(nc, in_maps, list(range(N_CORES)), trace=TRACE)
    LAST_RESULT = res
    LAST_EXEC_NS = res.exec_time_ns

    # ---- unpack ----
    graph_emb = np.empty((B, H), dtype=np.float32)
    attn = np.empty((N, NUM_HEADS), dtype=np.float32)
    for c in range(N_CORES):
        g = res.results[c]["g_out"]  # [2, 128, NSEG] slot-indexed
        a = res.results[c]["attn_out"]  # [R, Wa]
        segs_c = slot_seg[:, c]
        graph_emb[segs_c] = g.transpose(2, 0, 1).reshape(NSEG, H)
        aoff = 0
        for gidx in range(ngroups):
            Lg = Ls[gidx]
            for k in range(GS):
                slot = gidx * GS + k
                seg = segs_c[slot]
                ln = int(lens[seg])
                if ln:
                    s0 = int(starts[seg])
                    attn[s0 : s0 + ln] = a[4 * k : 4 * k + 4, aoff : aoff + ln].T
            aoff += Lg

    graph_emb[lens == 0] = 0.0
    return graph_emb, attn


# revision 22
# speedup vs baseline: 1.7662x; 1.0259x over previous
"""AttentionReadout (segment softmax attention pooling) on 8 Trainium2 cores.

Math (reference):
    logits = tanh(x @ W1 + b1) @ W2 + b2          # [N, 4]
    attn   = segment_softmax(logits, batch)       # [N, 4]
    xt     = x @ Wt + bt                          # [N, 256] -> [N, 4, 64]
    graph_emb = segment_sum(attn[:, :, None] * xt)  # [1024, 256]
    returns (graph_emb, attn)

Strategy:
  * Host packs nodes into a segment-padded layout: every segment gets L
    (>=512, multiple of 128) node slots, padding rows are zero and carry a
    -30000 logit mask.  1024 segments / 8 cores = 128 whole segments per
    core -> segment reductions are device-local, fixed-shape free-dim
    reductions.
  * Host pre-transposes x per segment to [hidden, L] so every matmul uses
    the weights as natural lhsT ([K, M]) and the activations as rhs
    ([K, N=L]) -- no on-device transposes.
  * All matmuls run as float32r (full fp32 data, 1 cycle/row at N>=256).
  * exp+segment-sum fuse into one scalar-engine activation (accum_out);
    (xt + bt) * attn and the graph sum fuse into one DVE
    scalar_tensor_tensor with accum_out.
  * Outputs: attn in transposed padded layout [nseg, 4, L]; graph
    embeddings as [2, 128, nseg] columns.  Host scatters back.
"""

import numpy as np
from contextlib import ExitStack

import concourse.bass as bass
import concourse.bacc as bacc
import concourse.tile as tile
from concourse import mybir
from concourse.bass_utils import run_bass_kernel_spmd

N_CORES = 8
H = 256
NUM_HEADS = 4
HEAD_DIM = H // NUM_HEADS
B = 1024
NSEG = B // N_CORES  # segments per core
NEG_MASK = -30000.0
GS = 8  # segments per softmax batch group

F32 = mybir.dt.float32
F32R = mybir.dt.float32r
AX = mybir.AxisListType.X
ALU = mybir.AluOpType
ACT = mybir.ActivationFunctionType

# compiled program cache, keyed by L (pad length per segment)
_programs: dict[int, bass.Bass] = {}

LAST_EXEC_NS = None
LAST_RESULT = None
TRACE = False


def _build_program(Ls, nseg: int = NSEG, gs: int = GS) -> bass.Bass:
    """Ls: per-group slot widths (len ngroups); group g holds gs segments,
    each padded to Ls[g] node slots."""
    ngroups = nseg // gs
    R = NUM_HEADS * gs
    Wx = int(sum(2 * Lg for Lg in Ls) * gs)  # xpt free width per partition
    Wm = int(sum(gs * Lg for Lg in Ls))  # mask free width
    Wa = int(sum(Lg for Lg in Ls))  # attn-out free width per group-row
    nc = bacc.Bacc()

    xpt = nc.dram_tensor("xpt", [128, Wx], F32R, kind="ExternalInput")
    maskt = nc.dram_tensor("maskt", [NUM_HEADS, Wm], F32, kind="ExternalInput")
    w1 = nc.dram_tensor("w1", [H, H], F32R, kind="ExternalInput")
    w2 = nc.dram_tensor("w2", [H, NUM_HEADS], F32R, kind="ExternalInput")
    wt = nc.dram_tensor("wt", [H, H], F32R, kind="ExternalInput")
    b1d = nc.dram_tensor("b1d", [H, 1], F32, kind="ExternalInput")
    b2d = nc.dram_tensor("b2d", [R, 1], F32, kind="ExternalInput")
    btd = nc.dram_tensor("btd", [H, 1], F32, kind="ExternalInput")
    e4d = nc.dram_tensor("e4d", [gs, 2, R, 128], F32R, kind="ExternalInput")

    attn_out = nc.dram_tensor("attn_out", [R, Wa], F32R, kind="ExternalOutput")
    g_out = nc.dram_tensor("g_out", [2, 128, nseg], F32, kind="ExternalOutput")

    Lmax = max(Ls)

    with tile.TileContext(nc) as tc, ExitStack() as ctx:
        const = ctx.enter_context(tc.tile_pool(name="const", bufs=1))
        sb = ctx.enter_context(tc.tile_pool(name="sb", bufs=2))
        ps = ctx.enter_context(tc.tile_pool(name="ps", bufs=1, space="PSUM"))

        # --- persistent weights / constants ---
        w1s0 = const.tile([128, H], F32R, tag="w1s0", name="w1s0")
        w1s1 = const.tile([128, H], F32R, tag="w1s1", name="w1s1")
        nc.sync.dma_start(w1s0[:], w1[0:128, :])
        nc.sync.dma_start(w1s1[:], w1[128:256, :])
        wts0 = const.tile([128, H], F32R, tag="wts0", name="wts0")
        wts1 = const.tile([128, H], F32R, tag="wts1", name="wts1")
        nc.sync.dma_start(wts0[:], wt[0:128, :])
        nc.sync.dma_start(wts1[:], wt[128:256, :])
        w2s0 = const.tile([128, NUM_HEADS], F32R, tag="w2s0", name="w2s0")
        w2s1 = const.tile([128, NUM_HEADS], F32R, tag="w2s1", name="w2s1")
        nc.sync.dma_start(w2s0[:], w2[0:128, :])
        nc.sync.dma_start(w2s1[:], w2[128:256, :])
        b1s = const.tile([128, 2], F32, tag="b1s", name="b1s")
        nc.sync.dma_start(b1s[:, 0:1], b1d[0:128, :])
        nc.sync.dma_start(b1s[:, 1:2], b1d[128:256, :])
        bts = const.tile([128, 2], F32, tag="bts", name="bts")
        nc.sync.dma_start(bts[:, 0:1], btd[0:128, :])
        nc.sync.dma_start(bts[:, 1:2], btd[128:256, :])
        b2s = const.tile([R, 1], F32, tag="b2s", name="b2s")
        nc.sync.dma_start(b2s[:], b2d[:, :])
        e4s = const.tile([R, gs * 2 * 128], F32R, tag="e4s", name="e4s")
        for k in range(gs):
            for j in range(2):
                blk = (k * 2 + j) * 128
                nc.sync.dma_start(e4s[:, blk : blk + 128], e4d[k, j, :, :])

        g0 = const.tile([128, nseg], F32, tag="g0", name="g0")
        g1 = const.tile([128, nseg], F32, tag="g1", name="g1")

        w1s = [w1s0, w1s1]
        wts = [wts0, wts1]
        w2s = [w2s0, w2s1]

        def phase2_seg(gp, kp, at_t, xts_list, Lg):
            sp = gp * gs + kp
            for j in range(2):
                blk = (kp * 2 + j) * 128
                rep = ps.tile([128, Lmax], F32, tag="rep", bufs=2, name="rep")
                nc.tensor.matmul(
                    rep[:, 0:Lg],
                    e4s[:, blk : blk + 128],
                    at_t[:, 0:Lg],
                    start=True,
                    stop=True,
                )
                wsc = sb.tile([128, Lmax], F32, tag=f"wsc{j}", name=f"wsc{j}")
                gcol = (g0 if j == 0 else g1)[:, sp : sp + 1]
                nc.vector.scalar_tensor_tensor(
                    wsc[:, 0:Lg],
                    xts_list[kp][j][:, 0:Lg],
                    1.0,
                    rep[:, 0:Lg],
                    op0=ALU.mult,
                    op1=ALU.mult,
                    accum_out=gcol,
                )

        xoff = 0  # running offset into xpt free dim
        moff = 0  # running offset into maskt free dim
        aoff = 0  # running offset into attn_out free dim
        prev = None
        for g in range(ngroups):
            Lg = int(Ls[g])
            lms = sb.tile([R, Lmax], F32, tag="lms", name="lms")
            mkg = sb.tile([NUM_HEADS, gs * Lmax], F32, tag="mkg", name="mkg")
            nc.sync.dma_start(
                mkg[:, 0 : gs * Lg], maskt[:, moff : moff + gs * Lg]
            )
            moff += gs * Lg

            xts_all = []
            for k in range(gs):
                s = g * gs + k
                xp = sb.tile([128, 2 * Lmax], F32R, tag="xp", bufs=4, name="xp")
                nc.sync.dma_start(
                    xp[:, 0 : 2 * Lg], xpt[:, xoff : xoff + 2 * Lg]
                )
                xoff += 2 * Lg
                xps = [xp[:, 0:Lg], xp[:, Lg : 2 * Lg]]

                hs = []
                for j in range(2):
                    ph = ps.tile([128, Lmax], F32, tag=f"ph{j}", name=f"ph{j}")
                    for i in range(2):
                        nc.tensor.matmul(
                            ph[:, 0:Lg],
                            w1s[i][:, j * 128 : (j + 1) * 128],
                            xps[i],
                            start=(i == 0),
                            stop=(i == 1),
                        )
                    hj = sb.tile([128, Lmax], F32R, tag=f"h{j}", name=f"h{j}")
                    nc.scalar.activation(
                        hj[:, 0:Lg], ph[:, 0:Lg], ACT.Tanh, bias=b1s[:, j : j + 1]
                    )
                    hs.append(hj)

                xts_pair = []
                for j in range(2):
                    px = ps.tile([128, Lmax], F32, tag=f"px{j}", name=f"px{j}")
                    for i in range(2):
                        nc.tensor.matmul(
                            px[:, 0:Lg],
                            wts[i][:, j * 128 : (j + 1) * 128],
                            xps[i],
                            start=(i == 0),
                            stop=(i == 1),
                        )
                    xts = sb.tile(
                        [128, Lmax], F32, tag=f"xts{j}", bufs=2 * gs + 2,
                        name=f"xts{j}",
                    )
                    nc.scalar.activation(
                        xts[:, 0:Lg], px[:, 0:Lg], ACT.Identity,
                        bias=bts[:, j : j + 1],
                    )
                    xts_pair.append(xts)
                xts_all.append(xts_pair)

                pl = ps.tile([NUM_HEADS, Lmax], F32, tag="pl", bufs=2, name="pl")
                for i in range(2):
                    nc.tensor.matmul(
                        pl[:, 0:Lg],
                        w2s[i][:],
                        hs[i][:, 0:Lg],
                        start=(i == 0),
                        stop=(i == 1),
                    )
                lml = sb.tile([NUM_HEADS, Lmax], F32, tag="lml", bufs=3, name="lml")
                nc.vector.tensor_add(
                    lml[:, 0:Lg], pl[:, 0:Lg], mkg[:, k * Lg : (k + 1) * Lg]
                )
                nc.sync.dma_start(lms[4 * k : 4 * k + 4, 0:Lg], lml[:, 0:Lg])

                if prev is not None:
                    phase2_seg(prev[0], k, prev[1], prev[2], prev[3])

            # ---- batched segment softmax over the whole group ----
            nmax = sb.tile([R, 1], F32, tag="nmax", name="nmax")
            nc.vector.reduce_max(nmax[:], lms[:, 0:Lg], axis=AX, negate=True)
            ebias = sb.tile([R, 1], F32, tag="ebias", name="ebias")
            nc.vector.tensor_add(ebias[:], nmax[:], b2s[:])
            ev = sb.tile([R, Lmax], F32, tag="ev", name="ev")
            ssum = sb.tile([R, 1], F32, tag="ssum", name="ssum")
            nc.scalar.activation(
                ev[:, 0:Lg], lms[:, 0:Lg], ACT.Exp, bias=ebias[:, 0:1],
                accum_out=ssum[:],
            )
            rinv = sb.tile([R, 1], F32, tag="rinv", name="rinv")
            nc.vector.reciprocal(rinv[:], ssum[:])
            at = sb.tile([R, Lmax], F32R, tag="at", name="at")
            nc.vector.tensor_scalar_mul(at[:, 0:Lg], ev[:, 0:Lg], rinv[:, 0:1])
            nc.sync.dma_start(attn_out[:, aoff : aoff + Lg], at[:, 0:Lg])
            aoff += Lg

            prev = (g, at, xts_all, Lg)

        for k in range(gs):
            phase2_seg(prev[0], k, prev[1], prev[2], prev[3])

        nc.sync.dma_start(g_out[0, :, :], g0[:])
        nc.sync.dma_start(g_out[1, :, :], g1[:])

    nc.finalize()
    return nc


def kernel(x, batch, W1, b1, W2, b2, Wt, bt):
    global LAST_EXEC_NS, LAST_RESULT

    x = np.ascontiguousarray(np.asarray(x, dtype=np.float32))
    batch = np.asarray(batch)
    N = x.shape[0]

    starts = np.searchsorted(batch, np.arange(B + 1)).astype(np.int64)
    lens = np.diff(starts)
    maxlen = int(lens.max())
    assert maxlen <= 512, f"segment length {maxlen} > 512 unsupported"

    # global sort by length; rank r -> (core r%8, slot r//8); group = slot//GS
    seg_perm = np.argsort(lens, kind="stable")  # seg ids in ascending length
    ngroups = NSEG // GS
    # group widths: max len among ranks in the group's 8*GS-range, >=256, mult of 16
    Ls = []
    for gidx in range(ngroups):
        hi = lens[seg_perm[(gidx + 1) * N_CORES * GS - 1]]
        Lg = max(256, int(np.ceil(hi / 8.0)) * 8)
        Ls.append(Lg)
    key = tuple(Ls)
    if key not in _programs:
        _programs.clear()
        _programs[key] = _build_program(Ls)
    nc = _programs[key]

    # node -> (segment, position-within-segment)
    pos = np.arange(N, dtype=np.int64) - starts[batch]

    # per (core, slot): seg id
    slot_seg = seg_perm.reshape(NSEG, N_CORES)  # [slot, core]

    W1 = np.ascontiguousarray(np.asarray(W1, dtype=np.float32))
    W2 = np.ascontiguousarray(np.asarray(W2, dtype=np.float32))
    Wt = np.ascontiguousarray(np.asarray(Wt, dtype=np.float32))
    b1c = np.ascontiguousarray(np.asarray(b1, dtype=np.float32).reshape(H, 1))
    b2c = np.ascontiguousarray(
        np.tile(np.asarray(b2, dtype=np.float32).reshape(1, NUM_HEADS), (GS, 1)).reshape(
            GS * NUM_HEADS, 1
        )
    )
    btc = np.ascontiguousarray(np.asarray(bt, dtype=np.float32).reshape(H, 1))
    e4 = np.zeros((GS, 2, GS * NUM_HEADS, 128), dtype=np.float32)
    for k in range(GS):
        for j in range(2):
            for hh in range(2):
                e4[k, j, 4 * k + 2 * j + hh, hh * HEAD_DIM : (hh + 1) * HEAD_DIM] = 1.0

    Wx = sum(2 * Lg for Lg in Ls) * GS
    Wm = sum(GS * Lg for Lg in Ls)
    Wa = sum(Lg for Lg in Ls)

    # pack per core
    xpt_cores = [np.zeros((128, Wx), np.float32) for _ in range(N_CORES)]
    mask_cores = [np.full((NUM_HEADS, Wm), NEG_MASK, np.float32) for _ in range(N_CORES)]
    xT = np.ascontiguousarray(x.T)  # [H, N]
    for c in range(N_CORES):
        xoff = 0
        moff = 0
        xc = xpt_cores[c]
        mc = mask_cores[c]
        for gidx in range(ngroups):
            Lg = Ls[gidx]
            for k in range(GS):
                slot = gidx * GS + k
                seg = slot_seg[slot, c]
                ln = int(lens[seg])
                s0 = int(starts[seg])
                if ln:
                    xc[:, xoff : xoff + ln] = xT[0:128, s0 : s0 + ln]
                    xc[:, xoff + Lg : xoff + Lg + ln] = xT[128:256, s0 : s0 + ln]
                    mc[:, moff + k * Lg : moff + k * Lg + ln] = 0.0
                xoff += 2 * Lg
            moff += GS * Lg

    in_maps = []
    for c in range(N_CORES):
        in_maps.append(
            dict(
                xpt=xpt_cores[c],
                maskt=mask_cores[c],
                w1=W1,
                w2=W2,
                wt=Wt,
                b1d=b1c,
                b2d=b2c,
                btd=btc,
                e4d=e4,
            )
        )

    res = run_bass_kernel_spmd(nc, in_maps, list(range(N_CORES)), trace=TRACE)
    LAST_RESULT = res
    LAST_EXEC_NS = res.exec_time_ns

    # ---- unpack ----
    graph_emb = np.empty((B, H), dtype=np.float32)
    attn = np.empty((N, NUM_HEADS), dtype=np.float32)
    for c in range(N_CORES):
        g = res.results[c]["g_out"]  # [2, 128, NSEG] slot-indexed
        a = res.results[c]["attn_out"]  # [R, Wa]
        segs_c = slot_seg[:, c]
        graph_emb[segs_c] = g.transpose(2, 0, 1).reshape(NSEG, H)
        aoff = 0
        for gidx in range(ngroups):
            Lg = Ls[gidx]
            for k in range(GS):
                slot = gidx * GS + k
                seg = segs_c[slot]
                ln = int(lens[seg])
                if ln:
                    s0 = int(starts[seg])
                    attn[s0 : s0 + ln] = a[4 * k : 4 * k + 4, aoff : aoff + ln].T
            aoff += Lg

    graph_emb[lens == 0] = 0.0
    return graph_emb, attn
